# revision 30
# baseline (speedup 1.0000x reference)
"""Trainium2 Bass kernel for a causal self-attention transformer layer.

Layer (PostNorm, eval):
    h  = MHA_causal(tgt); x = LN(tgt + h); out = LN(x + gelu(x@W1.T+b1)@W2.T+b2)
Shapes: B=2, N=2048, D=1024, H=16 (dk=64), FFN=4096.

All large GEMMs run in fp8-e4m3 with DoubleRow perf mode (two K-tiles
contracted per pass at 0.5 cycles/output-row). Weights are pre-scaled on
the host (x32 for Wq/Wk/Wv/Wo/W1, x128 for W2) so fp8 operands sit in the
e4m3 sweet spot; the inverse scales fold into activation `scale` args and
the softmax exp scale. The residual stream and LayerNorm stats stay fp32.

Distribution over 8 NeuronCores (core g: batch b=g//4, rank r=g%4):
  Stage A (head-parallel): QKV + causal attention for 4 local heads.
    q/k live as [h*32+d32, dk_half, token] so one DoubleRow matmul
    contracts the full dk=64 per head; AV pairs consecutive 128-key
    blocks. Softmax denominator rides as a 65th ones-column of V.
    exp widths are restricted to the causal region per diagonal block;
    multiplicative masks zero the remainder (incl. stale tile data).
  AllToAll (8 cores) converts head-sharded fp8 attention output into
    token-sharded; cross-batch slots are nulled by zero-padded Wo rows.
  Stage B (token-parallel): Wo + LN1 + W1/gelu + W2 + LN2 for 512
    tokens, W2 in two half-token passes so LN2 of half 0 overlaps the
    second pass on the PE.
"""

import numpy as np
import ml_dtypes

import concourse.bass as bass
import concourse.mybir as mybir
import concourse.tile as tile
from concourse.vector_clock import ScopedClock

BF16 = mybir.dt.bfloat16
F32 = mybir.dt.float32
F32R = mybir.dt.float32r
FP8 = mybir.dt.float8e4
AF = mybir.ActivationFunctionType
ALU = mybir.AluOpType
DR = mybir.MatmulPerfMode.DoubleRow

B, N, D, H, DK, FFN = 2, 2048, 1024, 16, 64, 4096
EPS = 1e-5
NCORES = 8
TPR = 4            # tensor-parallel ranks per batch
HL = H // TPR      # heads per core (4)
DVL = HL * DK      # local head width (256)
TS = N // TPR      # tokens per core in stage B (512)
NP = 128           # partitions
QT = 512           # q tile width
NKB = N // NP      # key blocks (16)
HTS = TS // 2      # stage-B half width (256)
SW = 32.0          # fp8 weight prescale (Wq/Wk/Wv/Wo/W1)
SW2 = 128.0        # fp8 weight prescale (W2)

# ---------------------------------------------------------------------------
# Walrus in this environment encodes at most ONE sync-wait per instruction.
# Patch Tile's exit drain and post-split every multi-wait instruction.
# ---------------------------------------------------------------------------

_wsplit = [0]


def _patched_drain_and_barrier(self, tick_clock, wait_clock):
    nc = self.nc
    probe = nc.sync.nop(nofuse=True)
    wait_clock.add_sem_waits(probe.ins, ScopedClock({None: tick_clock.global_clock}))
    si = probe.ins.sync_info
    waits = list(si.on_wait) if si is not None else []
    if waits:
        probe.ins.sync_info = mybir.SyncInfo(on_wait=[waits[0]], on_update=[])
        for w in waits[1:]:
            extra = nc.sync.nop(nofuse=True)
            extra.ins.sync_info = mybir.SyncInfo(on_wait=[w], on_update=[])
    nc.sync.drain()
    nc.all_engine_barrier()
    popped = nc._tile_sem_poison_stack.pop()
    assert popped is self._sem_poison
    nc.clear_and_free_semaphores(list(self.sems.allocated().values()))
    nc.all_engine_barrier()


tile.TileContext._drain_and_barrier = _patched_drain_and_barrier


def _split_multiwait_instructions(nc):
    for fn in nc.m.functions:
        for bb in fn.blocks:
            insts = bb.instructions
            if not any(
                i.sync_info is not None and len(i.sync_info.on_wait) > 1
                for i in insts
            ):
                continue
            new = []
            for inst in insts:
                si = inst.sync_info
                waits = list(si.on_wait) if si is not None else []
                if len(waits) > 1:
                    for w in waits[:-1]:
                        _wsplit[0] += 1
                        new.append(mybir.InstNoOp(
                            name=f"wsplit-{_wsplit[0]}",
                            engine=inst.engine,
                            sync_info=mybir.SyncInfo(on_wait=[w], on_update=[]),
                        ))
                    inst.sync_info = mybir.SyncInfo(
                        on_wait=[waits[-1]], on_update=list(si.on_update)
                    )
                new.append(inst)
            bb.instructions = new


# ---------------------------------------------------------------------------
# Program builder
# ---------------------------------------------------------------------------

def _build_program(for_sim=False, split=True):
    nc = bass.Bass("TRN2", target_bir_lowering=False, debug=False,
                   num_devices=NCORES)

    def din(name, shape, dt):
        return nc.dram_tensor(name, shape, dt, kind="ExternalInput").ap()

    xt_d = din("xt", [D, N], FP8)             # tgt[b].T (fp8)
    xtr_d = din("xtr", [D, TS], F32)          # residual slice (my tokens)
    wqt_d = din("wqt", [D, DVL], FP8)         # 32*Wq[local].T, dk-half-major
    wkt_d = din("wkt", [D, DVL], FP8)
    wvt_d = din("wvt", [D, DVL], FP8)         # 32*Wv[local].T, head-major
    wot_d = din("wot", [2 * D, D], FP8)       # 32*Wo.T zero-padded (A2A slots)
    w1t_d = din("w1t", [D, FFN], FP8)         # 32*W1.T
    w2t_d = din("w2t", [FFN, D], FP8)         # 128*W2.T
    bo_d = din("bo", [NP, D // NP], F32)
    b1_d = din("b1", [NP, FFN // NP], F32)
    b2_d = din("b2", [NP, D // NP], F32)
    mask_d = din("maskq", [NP, NP], FP8)
    out_d = nc.dram_tensor("out", [D, TS], F32, kind="ExternalOutput").ap()

    cc_in = [nc.dram_tensor(f"cc_in{i}", [NCORES * DVL, HTS], FP8).ap()
             for i in range(2)]
    cc_out = [nc.dram_tensor(f"cc_out{i}", [NCORES * DVL, HTS], FP8).ap()
              for i in range(2)]
    rb_d = nc.dram_tensor("rb_bounce", [16, QT], F32).ap()

    NDC = D // NP          # 8 feature chunks
    NP4 = NDC // 2         # 4 chunk pairs

    with tile.TileContext(nc, num_cores=NCORES) as tc:
        with tc.tile_pool(name="const", bufs=1) as cpool:
            onesf = cpool.tile([NP, NP], F32)
            nc.vector.memset(onesf[:], 1.0)
            ones128r = cpool.tile([1, NP], F32R)   # LN bcast lhsT
            nc.vector.tensor_copy(ones128r[:], onesf[0:1, :])
            onescol_r = cpool.tile([NP, 1], F32R)  # f32r stats lhsT
            nc.vector.tensor_copy(onescol_r[:], onesf[:, 0:1])
            bo_sb = cpool.tile([NP, D // NP], F32)
            nc.sync.dma_start(out=bo_sb[:], in_=bo_d[:])
            b1_sb = cpool.tile([NP, FFN // NP], F32)
            nc.sync.dma_start(out=b1_sb[:], in_=b1_d[:])
            b2_sb = cpool.tile([NP, D // NP], F32)
            nc.sync.dma_start(out=b2_sb[:], in_=b2_d[:])
            mask_sb = cpool.tile([NP, NP], FP8)
            warm = cpool.tile([1, 16], F32)
            nc.scalar.activation(warm[:], onesf[0:1, 0:16], AF.Exp)
            nc.scalar.activation(warm[:], onesf[0:1, 0:16],
                                 AF.Sigmoid if for_sim else AF.Gelu_apprx_tanh)

            # stage-B tiles allocated up front; their DMAs are emitted after
            # the attention-critical loads to fill spare DMA bandwidth
            prefetch = tc.tile_pool(name="prefetch", bufs=1)
            pf = prefetch.__enter__()
            wot_sb = pf.tile([NP, 2 * D // NP, D], FP8)
            xtr_sb = pf.tile([NP, D // NP, TS], F32)
            w2_sb = pf.tile([NP, FFN // NP // 2, 2, D], FP8)

            # ---------------- Stage A: QKV + attention (4 local heads) ----
            with tc.tile_pool(name="sa", bufs=1) as sa:
                wk_sb = sa.tile([NP, NDC, DVL], FP8)
                nc.sync.dma_start(
                    out=wk_sb[:], in_=wkt_d.rearrange("(c p) f -> p c f", p=NP))
                wq_sb = sa.tile([NP, NDC, DVL], FP8)
                nc.sync.dma_start(
                    out=wq_sb[:], in_=wqt_d.rearrange("(c p) f -> p c f", p=NP))
                xt_c = []
                for p4 in range(NP4):
                    xc = sa.tile([NP, 2, N], FP8, tag=f"xt{p4}", name=f"xt{p4}")
                    nc.sync.dma_start(
                        out=xc[:],
                        in_=xt_d[256 * p4:256 * (p4 + 1), :].rearrange(
                            "(two p) n -> p two n", p=NP))
                    xt_c.append(xc)
                wv_sb = sa.tile([NP, NDC, DVL], FP8)
                nc.sync.dma_start(
                    out=wv_sb[:], in_=wvt_d.rearrange("(c p) f -> p c f", p=NP))
                nc.sync.dma_start(out=mask_sb[:], in_=mask_d[:])
                nc.sync.dma_start(
                    out=wot_sb[:],
                    in_=wot_d.rearrange("(c p) f -> p c f", p=NP))
                nc.sync.dma_start(
                    out=xtr_sb[:],
                    in_=xtr_d.rearrange("(c p) t -> p c t", p=NP))
                nc.sync.dma_start(
                    out=w2_sb[:],
                    in_=w2t_d.rearrange("(f two p) d -> p f two d",
                                        p=NP, two=2))

                # natural layout: head h lives at partitions 64*(h%2) of
                # offset o=h//2 (bf16 scores keep PE slack, save drains)
                q_sb = sa.tile([NP, 2, N], BF16)
                k_sb = sa.tile([NP, 2, N], BF16)
                # head slots padded to 128 so DoubleRow ldweights pair
                # strides/offsets stay 64B-aligned; col 64 holds the
                # softmax-denominator ones column
                v_sb = sa.tile([NP, NKB, HL, NP], FP8)
                attn_all = sa.tile([DK, HL * N], FP8)
                attn_h = [attn_all[:, h * N:(h + 1) * N] for h in range(HL)]

                with tc.tile_pool(name="qkv_ps", bufs=1, space="PSUM") as qp:
                    # interleave k/q/v per q-tile so ACT k-drains, DVE
                    # q/v-drains, and PE matmuls overlap
                    for t in range(N // QT):
                        for o in range(2):
                            kps = qp.tile([NP, QT], F32, tag="kps", bufs=2,
                                          name=f"kps{o}_{t}")
                            for p4 in range(NP4):
                                nc.tensor.matmul(
                                    kps[:],
                                    wk_sb[:, 2 * p4:2 * p4 + 2,
                                          o * NP:(o + 1) * NP],
                                    xt_c[p4][:, :, t * QT:(t + 1) * QT],
                                    start=(p4 == 0), stop=(p4 == NP4 - 1),
                                    perf_mode=DR)
                            nc.scalar.activation(
                                k_sb[:, o, t * QT:(t + 1) * QT], kps[:],
                                AF.Copy)
                        for o in range(2):
                            qps = qp.tile([NP, QT], F32, tag="qps", bufs=2,
                                          name=f"qps{o}_{t}")
                            for p4 in range(NP4):
                                nc.tensor.matmul(
                                    qps[:],
                                    wq_sb[:, 2 * p4:2 * p4 + 2,
                                          o * NP:(o + 1) * NP],
                                    xt_c[p4][:, :, t * QT:(t + 1) * QT],
                                    start=(p4 == 0), stop=(p4 == NP4 - 1),
                                    perf_mode=DR)
                            nc.vector.tensor_copy(
                                q_sb[:, o, t * QT:(t + 1) * QT], qps[:])
                        for tb in range(4 * t, 4 * (t + 1)):
                            vps = qp.tile([NP, DVL], F32, tag="vps", bufs=2,
                                          name=f"vps{tb}")
                            for p4 in range(NP4):
                                nc.tensor.matmul(
                                    vps[:],
                                    xt_c[p4][:, :, tb * NP:(tb + 1) * NP],
                                    wv_sb[:, 2 * p4:2 * p4 + 2, :],
                                    start=(p4 == 0), stop=(p4 == NP4 - 1),
                                    perf_mode=DR)
                            vview = v_sb[:, tb, :, :]
                            nc.vector.tensor_copy(
                                vview[:, :, 0:DK],
                                vps[:].rearrange("p (h c) -> p h c", c=DK))
                            nc.vector.memset(vview[:, :, DK:DK + 1], 1.0)

                with tc.tile_pool(name="att_ps", bufs=1, space="PSUM") as ap:
                    pavs = [ap.tile([DK + 1, QT], F32, tag=f"pav{h}", bufs=1,
                                    name=f"pav{h}")
                            for h in range(HL)]
                    for j in range(N // QT):
                        npair = 2 * (j + 1)
                        for kp in range(npair):
                            kb0 = 2 * kp
                            wmin = max(kb0 - 4 * j, 0) * NP
                            for h in range(HL):
                                ps = ap.tile([NP, 2, QT], F32, tag="s",
                                             bufs=2)
                                o, hh = h // 2, h % 2
                                for i in range(2):
                                    kb = kb0 + i
                                    w0 = max(kb - 4 * j, 0) * NP
                                    nc.tensor.matmul(
                                        ps[:, i, w0:QT],
                                        k_sb[64 * hh:64 * (hh + 1), o,
                                             kb * NP:(kb + 1) * NP],
                                        q_sb[64 * hh:64 * (hh + 1), o,
                                             j * QT + w0:(j + 1) * QT],
                                        start=True, stop=True)
                                e_sb = sa.tile([NP, 2, QT], FP8, tag="e",
                                               bufs=6)
                                nc.scalar.activation(
                                    e_sb[:, :, wmin:QT], ps[:, :, wmin:QT],
                                    AF.Exp,
                                    scale=1.0 / (np.sqrt(DK) * SW * SW))
                                for i in range(2):
                                    di = kb0 + i - 4 * j
                                    if di >= 0:
                                        if di > 0:
                                            # exact zeros over the region
                                            # exp skipped (also clears stale
                                            # buffer bytes)
                                            nc.vector.memset(
                                                e_sb[:, i, 0:di * NP], 0.0)
                                        nc.vector.tensor_tensor(
                                            e_sb[:, i, di * NP:(di + 1) * NP],
                                            e_sb[:, i, di * NP:(di + 1) * NP],
                                            mask_sb[:], op=ALU.mult)
                                nc.tensor.matmul(
                                    pavs[h][0:DK + 1, :],
                                    v_sb[:, kb0:kb0 + 2, h, 0:DK + 1],
                                    e_sb[:],
                                    start=(kp == 0), stop=(kp == npair - 1),
                                    perf_mode=DR)
                        for h in range(HL):
                            site = j * HL + h
                            rr = sa.tile([NP, QT], F32, tag="rr", bufs=4)
                            nc.vector.reciprocal(
                                rr[64:65, :], pavs[h][DK:DK + 1, :])
                            nc.sync.dma_start(
                                out=rb_d[site:site + 1, :], in_=rr[64:65, :])
                            rbc = sa.tile([DK, QT], F32, tag="rbc", bufs=4)
                            nc.sync.dma_start(
                                out=rbc[:],
                                in_=rb_d[site:site + 1, :]
                                .partition_broadcast(DK))
                            nc.vector.tensor_tensor(
                                attn_h[h][:, j * QT:(j + 1) * QT],
                                pavs[h][0:DK, :], rbc[:], op=ALU.mult)
                        # stage chunk j (4 heads) into cc_in slots j, j+4
                        av4 = attn_all[:].rearrange("p (h t) -> p h t", h=HL)
                        for i in range(2):
                            src = av4[:, :, j * QT + i * HTS:
                                      j * QT + (i + 1) * HTS]
                            for s in (j, j + TPR):
                                dst = cc_in[i][DVL * s:DVL * (s + 1), :]
                                nc.sync.dma_start(
                                    out=dst.rearrange("(h p) t -> p h t",
                                                      p=DK),
                                    in_=src)

            for i in range(2):
                if for_sim:
                    # stand-in for the A2A so the single-core timeline sim
                    # runs: same bytes through the DMA path
                    nc.sync.dma_start(out=cc_out[i][:], in_=cc_in[i][:])
                else:
                    nc.gpsimd.collective_compute(
                        "AllToAll", ALU.bypass,
                        ins=[cc_in[i][:]], outs=[cc_out[i][:]],
                        replica_groups=[list(range(NCORES))],
                    )

            # ---------------- Stage B: Wo + LN1 + MLP + LN2 ---------------
            with tc.tile_pool(name="sbw", bufs=1) as sbw:
                x2f = sbw.tile([NP, NDC, TS], F32)
                x2q = sbw.tile([NP, NDC, TS], FP8)
                g_all = sbw.tile([NP, FFN // NP, TS], FP8)
                hs = [slice(0, HTS), slice(HTS, TS)]

                def wo_half(h01, wp, sbo):
                    hsl = hs[h01]
                    ao = sbo.tile([NP, 2 * D // NP, HTS], FP8, tag="ao",
                                  bufs=2, name=f"ao_{h01}")
                    for c4 in range(4):
                        nc.sync.dma_start(
                            out=ao[:, 4 * c4:4 * (c4 + 1), :],
                            in_=cc_out[h01][4 * NP * c4:4 * NP * (c4 + 1), :]
                            .rearrange("(c p) t -> p c t", p=NP))
                    sum1 = sbo.tile([NP, NDC, HTS], F32R, tag="sum1",
                                    bufs=2, name=f"sum1_{h01}")
                    for ob in range(NDC):
                        ph = wp.tile([NP, HTS], F32, tag="m", bufs=2)
                        for c2 in range(NDC):
                            nc.tensor.matmul(
                                ph[:], wot_sb[:, 2 * c2:2 * c2 + 2,
                                              ob * NP:(ob + 1) * NP],
                                ao[:, 2 * c2:2 * c2 + 2, :],
                                start=(c2 == 0), stop=(c2 == NDC - 1),
                                perf_mode=DR)
                        hb = sbo.tile([NP, HTS], F32, tag="hb", bufs=2,
                                      name=f"hb{h01}_{ob}")
                        nc.scalar.activation(hb[:], ph[:], AF.Identity,
                                             bias=bo_sb[:, ob:ob + 1],
                                             scale=1.0 / (SW * SW))
                        nc.vector.tensor_tensor(
                            sum1[:, ob, :], hb[:], xtr_sb[:, ob, hsl],
                            op=ALU.add)
                    return sum1

                def ln_sq(pool, src, W, pre=None):
                    sqb = pool.tile([NP, NDC, W], F32R, tag="ln_sq", bufs=2)
                    for ob in range(NDC):
                        if pre is not None:
                            pre(ob)
                        eng = nc.gpsimd if ob % 2 else nc.vector
                        eng.tensor_tensor(
                            sqb[:, ob, :], src[:, ob, :], src[:, ob, :],
                            op=ALU.mult)
                    return sqb

                def ln_rows(pool, lp, src, sqb, W):
                    pmu = lp.tile([1, W], F32, tag="statmu")
                    psq = lp.tile([1, W], F32, tag="statsq")
                    pmu, psq = pmu[:], psq[:]
                    for ob in range(NDC):
                        nc.tensor.matmul(pmu, onescol_r[:], src[:, ob, :],
                                         start=(ob == 0), stop=(ob == NDC - 1))
                        nc.tensor.matmul(psq, onescol_r[:], sqb[:, ob, :],
                                         start=(ob == 0), stop=(ob == NDC - 1))
                    rows = pool.tile([1, 4, W], F32, tag="ln_rows", bufs=2)
                    mu, ex2e, mu2, vr = (rows[:, i, :] for i in range(4))
                    nc.vector.tensor_scalar_mul(mu, pmu, 1.0 / D)
                    nc.vector.tensor_scalar(ex2e, psq, 1.0 / D, EPS,
                                            op0=ALU.mult, op1=ALU.add)
                    nc.vector.tensor_tensor(mu2, mu, mu, op=ALU.mult)
                    nc.vector.tensor_tensor(mu2, ex2e, mu2, op=ALU.subtract)
                    nc.vector.reciprocal(vr, mu2)
                    rowr = pool.tile([1, 2, W], F32R, tag="ln_rowr", bufs=2)
                    nc.scalar.activation(rowr[:, 0, :], vr, AF.Sqrt)
                    nc.vector.scalar_tensor_tensor(
                        rowr[:, 1, :], mu, -1.0, rowr[:, 0, :],
                        op0=ALU.mult, op1=ALU.mult)
                    return rowr

                def ln_finish(pool, lp, src, rowr, outf, outq, W, qoff=None):
                    pA = lp.tile([NP, W], F32, tag="bA")
                    pB = lp.tile([NP, W], F32, tag="bB")
                    pA, pB = pA[:], pB[:]
                    nc.tensor.matmul(pA, ones128r[:], rowr[:, 0, :],
                                     start=True, stop=True)
                    nc.tensor.matmul(pB, ones128r[:], rowr[:, 1, :],
                                     start=True, stop=True)
                    A_sb = pool.tile([NP, W], F32, tag="ln_A", bufs=2)
                    B_sb = pool.tile([NP, W], F32, tag="ln_B", bufs=2)
                    nc.scalar.activation(A_sb[:], pA, AF.Copy)
                    nc.scalar.activation(B_sb[:], pB, AF.Copy)
                    tmp = pool.tile([NP, NDC, W], F32, tag="ln_tmp", bufs=2)
                    for ob in range(NDC):
                        eng = nc.gpsimd if ob % 3 == 2 else nc.vector
                        eng.tensor_tensor(
                            tmp[:, ob, :], src[:, ob, :], A_sb[:],
                            op=ALU.mult)
                        nc.vector.tensor_tensor(
                            outf[ob], tmp[:, ob, :], B_sb[:], op=ALU.add)
                        if outq is not None:
                            eng2 = nc.gpsimd if ob % 3 == 1 else nc.vector
                            eng2.tensor_copy(outq[ob], outf[ob])

                with tc.tile_pool(name="sbo", bufs=1) as sbo, \
                     tc.tile_pool(name="ln1p", bufs=1) as lnp, \
                     tc.tile_pool(name="ln1_ps", bufs=1, space="PSUM") as lp, \
                     tc.tile_pool(name="wo_ps", bufs=1, space="PSUM") as wp:
                    for h01 in range(2):
                        sum1 = wo_half(h01, wp, sbo)
                        sqb = ln_sq(lnp, sum1, HTS)
                        rowr = ln_rows(lnp, lp, sum1, sqb, HTS)
                        ln_finish(lnp, lp, sum1, rowr,
                                  [x2f[:, ob, hs[h01]] for ob in range(NDC)],
                                  [x2q[:, ob, hs[h01]] for ob in range(NDC)],
                                  HTS)

                # W1 + gelu (full-width, fp8 DR over dc pairs)
                with tc.tile_pool(name="w1s", bufs=1) as w1s, \
                     tc.tile_pool(name="w1_ps", bufs=1, space="PSUM") as mp:
                    for fc in range(FFN // QT):
                        w1c = w1s.tile([NP, NDC, QT], FP8, tag="w1c",
                                       bufs=3, name=f"w1c{fc}")
                        nc.sync.dma_start(
                            out=w1c[:],
                            in_=w1t_d[:, fc * QT:(fc + 1) * QT].rearrange(
                                "(c p) f -> p c f", p=NP))
                        for fs in range(QT // NP):
                            fb = fc * (QT // NP) + fs
                            pm = mp.tile([NP, TS], F32, tag="pm", bufs=3)
                            for p4 in range(NP4):
                                nc.tensor.matmul(
                                    pm[:],
                                    w1c[:, 2 * p4:2 * p4 + 2,
                                        fs * NP:(fs + 1) * NP],
                                    x2q[:, 2 * p4:2 * p4 + 2, :],
                                    start=(p4 == 0), stop=(p4 == NP4 - 1),
                                    perf_mode=DR)
                            nc.scalar.activation(
                                g_all[:, fb, :], pm[:],
                                AF.Sigmoid if for_sim else AF.Gelu_apprx_tanh,
                                bias=b1_sb[:, fb:fb + 1], scale=1.0 / SW)

                # W2 in two half-token passes + LN2 overlapped
                with tc.tile_pool(name="w2p", bufs=1) as w2p, \
                     tc.tile_pool(name="ln2", bufs=1) as lnp2:
                    sum2 = w2p.tile([NP, NDC, TS], F32R)
                    yf = w2p.tile([NP, NDC, TS], F32)
                    yp_cm = tc.tile_pool(name="w2_ps", bufs=1, space="PSUM")
                    yp = yp_cm.__enter__()
                    for h01 in range(2):
                        hsl = hs[h01]
                        # one accumulator tile (= one PSUM bank) per output
                        # block: matmul start=True zeroes whole 2KB banks,
                        # so accumulators must never share a bank. 4 tags
                        # reused across ob-groups/halves keep stats banks
                        # free for the overlapped LN2.
                        ys_all = {}
                        for obg in range(2):
                            ys_t = [yp.tile([NP, HTS], F32, tag=f"ys{ob4}",
                                            bufs=1, name=f"ys{h01}_{obg}_{ob4}")
                                    for ob4 in range(4)]
                            for fp8i in range(FFN // NP // 2):
                                for ob4 in range(4):
                                    ob = obg * 4 + ob4
                                    nc.tensor.matmul(
                                        ys_t[ob4][:],
                                        w2_sb[:, fp8i, :,
                                              ob * NP:(ob + 1) * NP],
                                        g_all[:, 2 * fp8i:2 * fp8i + 2, hsl],
                                        start=(fp8i == 0),
                                        stop=(fp8i == FFN // NP // 2 - 1),
                                        perf_mode=DR)
                            for ob4 in range(4):
                                ys_all[obg * 4 + ob4] = ys_t[ob4]

                        def w2_tail(ob, h01=h01, ys_all=ys_all):
                            mb = w2p.tile([NP, HTS], F32, tag="mb", bufs=2,
                                          name=f"mb{h01}_{ob}")
                            nc.scalar.activation(
                                mb[:], ys_all[ob][:], AF.Identity,
                                bias=b2_sb[:, ob:ob + 1], scale=1.0 / SW2)
                            nc.vector.tensor_tensor(
                                sum2[:, ob, hs[h01]], mb[:],
                                x2f[:, ob, hs[h01]], op=ALU.add)

                        src = sum2[:, :, hsl]
                        sqb = ln_sq(lnp2, src, HTS, pre=w2_tail)
                        with tc.tile_pool(name=f"ln2_ps{h01}", bufs=1,
                                          space="PSUM") as lp2:
                            rowr = ln_rows(lnp2, lp2, src, sqb, HTS)
                            ln_finish(lnp2, lp2, src, rowr,
                                      [yf[:, ob, hsl] for ob in range(NDC)],
                                      None, HTS)
                        for ob in range(NDC):
                            nc.sync.dma_start(
                                out=out_d[ob * NP:(ob + 1) * NP, hsl],
                                in_=yf[:, ob, hsl])
                    yp_cm.__exit__(None, None, None)
            prefetch.__exit__(None, None, None)

    if split:
        _split_multiwait_instructions(nc)
    return nc


# ---------------------------------------------------------------------------
# Cached PJRT runner (mirrors bass2jax.run_bass_via_pjrt multi-core path but
# keeps the jitted callable so repeat calls don't recompile).
# ---------------------------------------------------------------------------

_RUNNER = None


def _make_runner(nc):
    import jax
    from jax.sharding import Mesh, PartitionSpec
    from jax.experimental.shard_map import shard_map
    from concourse import bass2jax

    bass2jax.install_neuronx_cc_hook()
    partition_name = (nc.partition_id_tensor.name
                      if nc.partition_id_tensor else None)
    in_names, out_names, out_avals = [], [], []
    for alloc in nc.m.functions[0].allocations:
        if not isinstance(alloc, mybir.MemoryLocationSet):
            continue
        name = alloc.memorylocations[0].name
        if alloc.kind == "ExternalInput":
            if name != partition_name:
                in_names.append(name)
        elif alloc.kind == "ExternalOutput":
            out_names.append(name)
            out_avals.append(jax.core.ShapedArray(
                tuple(alloc.tensor_shape), mybir.dt.np(alloc.dtype)))
    n_params = len(in_names)
    all_in_names = list(in_names) + list(out_names)
    if partition_name is not None:
        all_in_names.append(partition_name)
    donate = tuple(range(n_params, n_params + len(out_names)))

    def _body(*args):
        operands = list(args)
        if partition_name is not None:
            operands.append(bass2jax.partition_id_tensor())
        outs = bass2jax._bass_exec_p.bind(
            *operands,
            out_avals=tuple(out_avals),
            in_names=tuple(all_in_names),
            out_names=tuple(out_names),
            lowering_input_output_aliases=(),
            sim_require_finite=True,
            sim_require_nnan=True,
            nc=nc,
        )
        return tuple(outs)

    devices = jax.devices()[:NCORES]
    mesh = Mesh(np.asarray(devices), ("core",))
    specs = (PartitionSpec("core"),) * (n_params + len(out_names))
    sharded = jax.jit(
        shard_map(_body, mesh=mesh, in_specs=specs,
                  out_specs=(PartitionSpec("core"),) * len(out_names),
                  check_rep=False),
        donate_argnums=donate, keep_unused=True)

    from jax.sharding import NamedSharding
    shard = NamedSharding(mesh, PartitionSpec("core"))

    def prepare(in_maps):
        per_core = [[np.asarray(m[name]) for name in in_names]
                    for m in in_maps]
        concat_in = [np.concatenate([per_core[c][i] for c in range(NCORES)],
                                    axis=0) for i in range(n_params)]
        return [jax.device_put(a, shard) for a in concat_in]

    def run_prepared(dev_in, materialize=True):
        concat_zeros = [np.zeros((NCORES * a.shape[0], *a.shape[1:]), a.dtype)
                        for a in out_avals]
        out_arrs = sharded(*dev_in, *concat_zeros)
        if not materialize:
            jax.block_until_ready(out_arrs)
            return None
        return [
            {name: np.asarray(out_arrs[i]).reshape(
                NCORES, *out_avals[i].shape)[c]
             for i, name in enumerate(out_names)}
            for c in range(NCORES)
        ]

    def run_prepared_async(dev_in):
        concat_zeros = [np.zeros((NCORES * a.shape[0], *a.shape[1:]), a.dtype)
                        for a in out_avals]
        return sharded(*dev_in, *concat_zeros)

    def run(in_maps, materialize=True):
        return run_prepared(prepare(in_maps), materialize)

    run.prepare = prepare
    run.run_prepared = run_prepared
    run.run_prepared_async = run_prepared_async
    return run


def _get_runner():
    global _RUNNER
    if _RUNNER is None:
        nc = _build_program()
        _RUNNER = _make_runner(nc)
    return _RUNNER


# ---------------------------------------------------------------------------
# Host-side sharding / gathering
# ---------------------------------------------------------------------------

def _fp8(a):
    return np.ascontiguousarray(a).astype(ml_dtypes.float8_e4m3)


def make_in_maps(tgt, tgt_mask, tgt_key_pad_mask, Wq, Wk, Wv, Wo, bo,
                 W1, b1, W2, b2, g1, beta1, g2, beta2):
    causal = np.triu(np.ones((N, N), bool), k=1)
    if not (np.array_equal(np.asarray(tgt_mask), causal)
            and not np.asarray(tgt_key_pad_mask).any()
            and np.allclose(np.asarray(g1), 1) and np.allclose(np.asarray(g2), 1)
            and np.allclose(np.asarray(beta1), 0)
            and np.allclose(np.asarray(beta2), 0)):
        return None  # unsupported masking/affine -> numpy fallback

    tgt = np.asarray(tgt, np.float32)
    Wq, Wk, Wv, Wo = (np.asarray(a, np.float32) for a in (Wq, Wk, Wv, Wo))
    W1, W2 = np.asarray(W1, np.float32), np.asarray(W2, np.float32)
    bo, b1, b2 = (np.asarray(a, np.float32) for a in (bo, b1, b2))

    # single [128, 128] diagonal-triangle mask (same for every block)
    maskq = (np.arange(NP)[:, None] <= np.arange(NP)[None, :]).astype(
        np.float32)

    w1t = _fp8(SW * W1.T)
    w2t = _fp8(SW2 * W2.T)
    bo_c = np.ascontiguousarray(bo.reshape(D // NP, NP).T)
    b1_c = np.ascontiguousarray(b1.reshape(FFN // NP, NP).T)
    b2_c = np.ascontiguousarray(b2.reshape(D // NP, NP).T)
    maskq8 = _fp8(maskq)

    xt_b = [_fp8(tgt[b].T) for b in range(B)]
    in_maps = []
    for g in range(NCORES):
        b, r = g // TPR, g % TPR
        sl = slice(DVL * r, DVL * (r + 1))
        wot_ext = np.zeros((2 * D, D), np.float32)
        wot_ext[D * b:D * (b + 1), :] = SW * Wo.T
        in_maps.append({
            "xt": xt_b[b],
            "xtr": np.ascontiguousarray(tgt[b].T[:, TS * r:TS * (r + 1)]),
            "wqt": _fp8(SW * Wq[sl, :].T),
            "wkt": _fp8(SW * Wk[sl, :].T),
            "wvt": _fp8(SW * Wv[sl, :].T),
            "wot": _fp8(wot_ext),
            "w1t": w1t,
            "w2t": w2t,
            "bo": bo_c,
            "b1": b1_c,
            "b2": b2_c,
            "maskq": maskq8,
        })
    return in_maps


def _numpy_reference(tgt, tgt_mask, tgt_key_pad_mask, Wq, Wk, Wv, Wo, bo,
                     W1, b1, W2, b2, g1, beta1, g2, beta2):
    def ln(x, g, b):
        mu = x.mean(-1, keepdims=True)
        var = ((x - mu) ** 2).mean(-1, keepdims=True)
        return (x - mu) / np.sqrt(var + EPS) * g + b

    x = np.asarray(tgt, np.float64)
    b_, n, d = x.shape
    dk = d // H
    q = (x @ Wq.T).reshape(b_, n, H, dk).transpose(0, 2, 1, 3)
    k = (x @ Wk.T).reshape(b_, n, H, dk).transpose(0, 2, 1, 3)
    v = (x @ Wv.T).reshape(b_, n, H, dk).transpose(0, 2, 1, 3)
    s = np.einsum("bhqd,bhkd->bhqk", q, k) / np.sqrt(dk)
    mask = np.asarray(tgt_mask)[None, None] | \
        np.asarray(tgt_key_pad_mask)[:, None, None, :]
    s = np.where(mask, -np.inf, s)
    s = s - s.max(-1, keepdims=True)
    e = np.exp(s)
    att = e / e.sum(-1, keepdims=True)
    o = np.einsum("bhqk,bhkd->bhqd", att, v).transpose(0, 2, 1, 3).reshape(
        b_, n, d)
    h = o @ Wo.T + bo
    x1 = ln(x + h, g1, beta1)
    gl = x1 @ W1.T + b1
    gl = 0.5 * gl * (1 + np.tanh(np.sqrt(2 / np.pi) * (gl + 0.044715 * gl**3)))
    m = gl @ W2.T + b2
    return ln(x1 + m, g2, beta2).astype(np.float32)


def kernel(**inputs):
    in_maps = make_in_maps(**inputs)
    if in_maps is None:
        return _numpy_reference(**inputs)
    run = _get_runner()
    results = run(in_maps)
    out = np.empty((B, N, D), np.float32)
    for g in range(NCORES):
        b, r = g // TPR, g % TPR
        out[b, TS * r:TS * (r + 1), :] = results[g]["out"].T
    return out


# revision 32
# speedup vs baseline: 1.0420x; 1.0420x over previous
"""Trainium2 Bass kernel for a causal self-attention transformer layer.

Layer (PostNorm, eval):
    h  = MHA_causal(tgt); x = LN(tgt + h); out = LN(x + gelu(x@W1.T+b1)@W2.T+b2)
Shapes: B=2, N=2048, D=1024, H=16 (dk=64), FFN=4096.

All large GEMMs run in fp8-e4m3 with DoubleRow perf mode (two K-tiles
contracted per pass at 0.5 cycles/output-row). Weights are pre-scaled on
the host (x32 for Wq/Wk/Wv/Wo/W1, x128 for W2) so fp8 operands sit in the
e4m3 sweet spot; the inverse scales fold into activation `scale` args and
the softmax exp scale. The residual stream and LayerNorm stats stay fp32.

Distribution over 8 NeuronCores (core g: batch b=g//4, rank r=g%4):
  Stage A (head-parallel): QKV + causal attention for 4 local heads.
    q/k live as [h*32+d32, dk_half, token] so one DoubleRow matmul
    contracts the full dk=64 per head; AV pairs consecutive 128-key
    blocks. Softmax denominator rides as a 65th ones-column of V.
    exp widths are restricted to the causal region per diagonal block;
    multiplicative masks zero the remainder (incl. stale tile data).
  AllToAll (8 cores) converts head-sharded fp8 attention output into
    token-sharded; cross-batch slots are nulled by zero-padded Wo rows.
  Stage B (token-parallel): Wo + LN1 + W1/gelu + W2 + LN2 for 512
    tokens, W2 in two half-token passes so LN2 of half 0 overlaps the
    second pass on the PE.
"""

import numpy as np
import ml_dtypes

import concourse.bass as bass
import concourse.mybir as mybir
import concourse.tile as tile
from concourse.vector_clock import ScopedClock

BF16 = mybir.dt.bfloat16
F32 = mybir.dt.float32
F32R = mybir.dt.float32r
FP8 = mybir.dt.float8e4
AF = mybir.ActivationFunctionType
ALU = mybir.AluOpType
DR = mybir.MatmulPerfMode.DoubleRow

B, N, D, H, DK, FFN = 2, 2048, 1024, 16, 64, 4096
EPS = 1e-5
NCORES = 8
TPR = 4            # tensor-parallel ranks per batch
HL = H // TPR      # heads per core (4)
DVL = HL * DK      # local head width (256)
TS = N // TPR      # tokens per core in stage B (512)
NP = 128           # partitions
QT = 512           # q tile width
NKB = N // NP      # key blocks (16)
HTS = TS // 2      # stage-B half width (256)
SW = 32.0          # fp8 weight prescale (Wq/Wk/Wv/Wo/W1)
SW2 = 128.0        # fp8 weight prescale (W2)

# ---------------------------------------------------------------------------
# Walrus in this environment encodes at most ONE sync-wait per instruction.
# Patch Tile's exit drain and post-split every multi-wait instruction.
# ---------------------------------------------------------------------------

_wsplit = [0]


def _patched_drain_and_barrier(self, tick_clock, wait_clock):
    nc = self.nc
    probe = nc.sync.nop(nofuse=True)
    wait_clock.add_sem_waits(probe.ins, ScopedClock({None: tick_clock.global_clock}))
    si = probe.ins.sync_info
    waits = list(si.on_wait) if si is not None else []
    if waits:
        probe.ins.sync_info = mybir.SyncInfo(on_wait=[waits[0]], on_update=[])
        for w in waits[1:]:
            extra = nc.sync.nop(nofuse=True)
            extra.ins.sync_info = mybir.SyncInfo(on_wait=[w], on_update=[])
    nc.sync.drain()
    nc.all_engine_barrier()
    popped = nc._tile_sem_poison_stack.pop()
    assert popped is self._sem_poison
    nc.clear_and_free_semaphores(list(self.sems.allocated().values()))
    nc.all_engine_barrier()


tile.TileContext._drain_and_barrier = _patched_drain_and_barrier


def _split_multiwait_instructions(nc):
    for fn in nc.m.functions:
        for bb in fn.blocks:
            insts = bb.instructions
            if not any(
                i.sync_info is not None and len(i.sync_info.on_wait) > 1
                for i in insts
            ):
                continue
            new = []
            for inst in insts:
                si = inst.sync_info
                waits = list(si.on_wait) if si is not None else []
                if len(waits) > 1:
                    for w in waits[:-1]:
                        _wsplit[0] += 1
                        new.append(mybir.InstNoOp(
                            name=f"wsplit-{_wsplit[0]}",
                            engine=inst.engine,
                            sync_info=mybir.SyncInfo(on_wait=[w], on_update=[]),
                        ))
                    inst.sync_info = mybir.SyncInfo(
                        on_wait=[waits[-1]], on_update=list(si.on_update)
                    )
                new.append(inst)
            bb.instructions = new


# ---------------------------------------------------------------------------
# Program builder
# ---------------------------------------------------------------------------

def _build_program(for_sim=False, split=True):
    nc = bass.Bass("TRN2", target_bir_lowering=False, debug=False,
                   num_devices=NCORES)

    def din(name, shape, dt):
        return nc.dram_tensor(name, shape, dt, kind="ExternalInput").ap()

    xt_d = din("xt", [D, N], FP8)             # tgt[b].T (fp8)
    xtr_d = din("xtr", [D, TS], F32)          # residual slice (my tokens)
    wqt_d = din("wqt", [D, DVL], FP8)         # 32*Wq[local].T, dk-half-major
    wkt_d = din("wkt", [D, DVL], FP8)
    wvt_d = din("wvt", [D, DVL], FP8)         # 32*Wv[local].T, head-major
    wot_d = din("wot", [2 * D, D], FP8)       # 32*Wo.T zero-padded (A2A slots)
    w1t_d = din("w1t", [D, FFN], FP8)         # 32*W1.T
    w2t_d = din("w2t", [FFN, D], FP8)         # 128*W2.T
    bo_d = din("bo", [NP, D // NP], F32)
    b1_d = din("b1", [NP, FFN // NP], F32)
    b2_d = din("b2", [NP, D // NP], F32)
    mask_d = din("maskq", [NP, NP], FP8)
    out_d = nc.dram_tensor("out", [D, TS], F32, kind="ExternalOutput").ap()

    cc_in = [nc.dram_tensor(f"cc_in{i}", [NCORES * DVL, HTS], FP8).ap()
             for i in range(2)]
    cc_out = [nc.dram_tensor(f"cc_out{i}", [NCORES * DVL, HTS], FP8).ap()
              for i in range(2)]
    rb_d = nc.dram_tensor("rb_bounce", [16, QT], F32).ap()

    NDC = D // NP          # 8 feature chunks
    NP4 = NDC // 2         # 4 chunk pairs

    with tile.TileContext(nc, num_cores=NCORES) as tc:
        with tc.tile_pool(name="const", bufs=1) as cpool:
            onesf = cpool.tile([NP, NP], F32)
            nc.vector.memset(onesf[:], 1.0)
            ones128r = cpool.tile([1, NP], F32R)   # LN bcast lhsT
            nc.vector.tensor_copy(ones128r[:], onesf[0:1, :])
            onescol_r = cpool.tile([NP, 1], F32R)  # f32r stats lhsT
            nc.vector.tensor_copy(onescol_r[:], onesf[:, 0:1])
            bo_sb = cpool.tile([NP, D // NP], F32)
            nc.sync.dma_start(out=bo_sb[:], in_=bo_d[:])
            b1_sb = cpool.tile([NP, FFN // NP], F32)
            nc.sync.dma_start(out=b1_sb[:], in_=b1_d[:])
            b2_sb = cpool.tile([NP, D // NP], F32)
            nc.sync.dma_start(out=b2_sb[:], in_=b2_d[:])
            mask_sb = cpool.tile([NP, NP], FP8)
            warm = cpool.tile([1, 16], F32)
            nc.scalar.activation(warm[:], onesf[0:1, 0:16], AF.Exp)
            nc.scalar.activation(warm[:], onesf[0:1, 0:16],
                                 AF.Sigmoid if for_sim else AF.Gelu_apprx_tanh)

            # stage-B tiles allocated up front; their DMAs are emitted after
            # the attention-critical loads to fill spare DMA bandwidth
            prefetch = tc.tile_pool(name="prefetch", bufs=1)
            pf = prefetch.__enter__()
            wot_sb = pf.tile([NP, 2 * D // NP, D], FP8)
            xtr_sb = pf.tile([NP, D // NP, TS], F32)
            w2_sb = pf.tile([NP, FFN // NP // 2, 2, D], FP8)

            # ---------------- Stage A: QKV + attention (4 local heads) ----
            with tc.tile_pool(name="sa", bufs=1) as sa:
                wk_sb = sa.tile([NP, NDC, DVL], FP8)
                nc.sync.dma_start(
                    out=wk_sb[:], in_=wkt_d.rearrange("(c p) f -> p c f", p=NP))
                wq_sb = sa.tile([NP, NDC, DVL], FP8)
                nc.sync.dma_start(
                    out=wq_sb[:], in_=wqt_d.rearrange("(c p) f -> p c f", p=NP))
                xt_c = []
                for p4 in range(NP4):
                    xc = sa.tile([NP, 2, N], FP8, tag=f"xt{p4}", name=f"xt{p4}")
                    nc.sync.dma_start(
                        out=xc[:],
                        in_=xt_d[256 * p4:256 * (p4 + 1), :].rearrange(
                            "(two p) n -> p two n", p=NP))
                    xt_c.append(xc)
                wv_sb = sa.tile([NP, NDC, DVL], FP8)
                nc.sync.dma_start(
                    out=wv_sb[:], in_=wvt_d.rearrange("(c p) f -> p c f", p=NP))
                nc.sync.dma_start(out=mask_sb[:], in_=mask_d[:])
                nc.sync.dma_start(
                    out=wot_sb[:],
                    in_=wot_d.rearrange("(c p) f -> p c f", p=NP))
                nc.sync.dma_start(
                    out=xtr_sb[:],
                    in_=xtr_d.rearrange("(c p) t -> p c t", p=NP))
                nc.sync.dma_start(
                    out=w2_sb[:],
                    in_=w2t_d.rearrange("(f two p) d -> p f two d",
                                        p=NP, two=2))

                # natural layout: head h lives at partitions 64*(h%2) of
                # offset o=h//2 (bf16 scores keep PE slack, save drains)
                q_sb = sa.tile([NP, 2, N], BF16)
                k_sb = sa.tile([NP, 2, N], BF16)
                # head slots padded to 128 so DoubleRow ldweights pair
                # strides/offsets stay 64B-aligned; col 64 holds the
                # softmax-denominator ones column
                v_sb = sa.tile([NP, NKB, HL, NP], FP8)
                attn_all = sa.tile([DK, HL * N], FP8)
                attn_h = [attn_all[:, h * N:(h + 1) * N] for h in range(HL)]

                with tc.tile_pool(name="qkv_ps", bufs=1, space="PSUM") as qp:
                    # interleave k/q/v per q-tile so ACT k-drains, DVE
                    # q/v-drains, and PE matmuls overlap
                    for t in range(N // QT):
                        for o in range(2):
                            kps = qp.tile([NP, QT], F32, tag="kps", bufs=2,
                                          name=f"kps{o}_{t}")
                            for p4 in range(NP4):
                                nc.tensor.matmul(
                                    kps[:],
                                    wk_sb[:, 2 * p4:2 * p4 + 2,
                                          o * NP:(o + 1) * NP],
                                    xt_c[p4][:, :, t * QT:(t + 1) * QT],
                                    start=(p4 == 0), stop=(p4 == NP4 - 1),
                                    perf_mode=DR)
                            nc.scalar.activation(
                                k_sb[:, o, t * QT:(t + 1) * QT], kps[:],
                                AF.Copy)
                        for o in range(2):
                            qps = qp.tile([NP, QT], F32, tag="qps", bufs=2,
                                          name=f"qps{o}_{t}")
                            for p4 in range(NP4):
                                nc.tensor.matmul(
                                    qps[:],
                                    wq_sb[:, 2 * p4:2 * p4 + 2,
                                          o * NP:(o + 1) * NP],
                                    xt_c[p4][:, :, t * QT:(t + 1) * QT],
                                    start=(p4 == 0), stop=(p4 == NP4 - 1),
                                    perf_mode=DR)
                            nc.vector.tensor_copy(
                                q_sb[:, o, t * QT:(t + 1) * QT], qps[:])
                        for tb in range(4 * t, 4 * (t + 1)):
                            vps = qp.tile([NP, DVL], F32, tag="vps", bufs=2,
                                          name=f"vps{tb}")
                            for p4 in range(NP4):
                                nc.tensor.matmul(
                                    vps[:],
                                    xt_c[p4][:, :, tb * NP:(tb + 1) * NP],
                                    wv_sb[:, 2 * p4:2 * p4 + 2, :],
                                    start=(p4 == 0), stop=(p4 == NP4 - 1),
                                    perf_mode=DR)
                            vview = v_sb[:, tb, :, :]
                            nc.scalar.activation(
                                vview[:, :, 0:DK],
                                vps[:].rearrange("p (h c) -> p h c", c=DK),
                                AF.Copy)
                            nc.vector.memset(vview[:, :, DK:DK + 1], 1.0)

                with tc.tile_pool(name="att_ps", bufs=1, space="PSUM") as ap:
                    pavs = [ap.tile([DK + 1, QT], F32, tag=f"pav{h}", bufs=1,
                                    name=f"pav{h}")
                            for h in range(HL)]
                    for j in range(N // QT):
                        npair = 2 * (j + 1)
                        for kp in range(npair):
                            kb0 = 2 * kp
                            wmin = max(kb0 - 4 * j, 0) * NP
                            for h in range(HL):
                                ps = ap.tile([NP, 2, QT], F32, tag="s",
                                             bufs=2)
                                o, hh = h // 2, h % 2
                                for i in range(2):
                                    kb = kb0 + i
                                    w0 = max(kb - 4 * j, 0) * NP
                                    nc.tensor.matmul(
                                        ps[:, i, w0:QT],
                                        k_sb[64 * hh:64 * (hh + 1), o,
                                             kb * NP:(kb + 1) * NP],
                                        q_sb[64 * hh:64 * (hh + 1), o,
                                             j * QT + w0:(j + 1) * QT],
                                        start=True, stop=True)
                                e_sb = sa.tile([NP, 2, QT], FP8, tag="e",
                                               bufs=6)
                                nc.scalar.activation(
                                    e_sb[:, :, wmin:QT], ps[:, :, wmin:QT],
                                    AF.Exp,
                                    scale=1.0 / (np.sqrt(DK) * SW * SW))
                                for i in range(2):
                                    di = kb0 + i - 4 * j
                                    if di >= 0:
                                        if di > 0:
                                            # exact zeros over the region
                                            # exp skipped (also clears stale
                                            # buffer bytes)
                                            nc.gpsimd.memset(
                                                e_sb[:, i, 0:di * NP], 0.0)
                                        nc.vector.tensor_tensor(
                                            e_sb[:, i, di * NP:(di + 1) * NP],
                                            e_sb[:, i, di * NP:(di + 1) * NP],
                                            mask_sb[:], op=ALU.mult)
                                nc.tensor.matmul(
                                    pavs[h][0:DK + 1, :],
                                    v_sb[:, kb0:kb0 + 2, h, 0:DK + 1],
                                    e_sb[:],
                                    start=(kp == 0), stop=(kp == npair - 1),
                                    perf_mode=DR)
                        for h in range(HL):
                            site = j * HL + h
                            rr = sa.tile([NP, QT], F32, tag="rr", bufs=4)
                            nc.vector.reciprocal(
                                rr[64:65, :], pavs[h][DK:DK + 1, :])
                            nc.sync.dma_start(
                                out=rb_d[site:site + 1, :], in_=rr[64:65, :])
                            rbc = sa.tile([DK, QT], F32, tag="rbc", bufs=4)
                            nc.sync.dma_start(
                                out=rbc[:],
                                in_=rb_d[site:site + 1, :]
                                .partition_broadcast(DK))
                            nc.vector.tensor_tensor(
                                attn_h[h][:, j * QT:(j + 1) * QT],
                                pavs[h][0:DK, :], rbc[:], op=ALU.mult)
                        # stage chunk j (4 heads) into cc_in slots j, j+4
                        av4 = attn_all[:].rearrange("p (h t) -> p h t", h=HL)
                        for i in range(2):
                            src = av4[:, :, j * QT + i * HTS:
                                      j * QT + (i + 1) * HTS]
                            for s in (j, j + TPR):
                                dst = cc_in[i][DVL * s:DVL * (s + 1), :]
                                nc.sync.dma_start(
                                    out=dst.rearrange("(h p) t -> p h t",
                                                      p=DK),
                                    in_=src)

            for i in range(2):
                if for_sim:
                    # stand-in for the A2A so the single-core timeline sim
                    # runs: same bytes through the DMA path
                    nc.sync.dma_start(out=cc_out[i][:], in_=cc_in[i][:])
                else:
                    nc.gpsimd.collective_compute(
                        "AllToAll", ALU.bypass,
                        ins=[cc_in[i][:]], outs=[cc_out[i][:]],
                        replica_groups=[list(range(NCORES))],
                    )

            # ---------------- Stage B: Wo + LN1 + MLP + LN2 ---------------
            with tc.tile_pool(name="sbw", bufs=1) as sbw:
                x2f = sbw.tile([NP, NDC, TS], F32)
                x2q = sbw.tile([NP, NDC, TS], FP8)
                g_all = sbw.tile([NP, FFN // NP, TS], FP8)
                hs = [slice(0, HTS), slice(HTS, TS)]

                def wo_half(h01, wp, sbo):
                    hsl = hs[h01]
                    ao = sbo.tile([NP, 2 * D // NP, HTS], FP8, tag="ao",
                                  bufs=2, name=f"ao_{h01}")
                    for c4 in range(4):
                        nc.sync.dma_start(
                            out=ao[:, 4 * c4:4 * (c4 + 1), :],
                            in_=cc_out[h01][4 * NP * c4:4 * NP * (c4 + 1), :]
                            .rearrange("(c p) t -> p c t", p=NP))
                    sum1 = sbo.tile([NP, NDC, HTS], F32R, tag="sum1",
                                    bufs=2, name=f"sum1_{h01}")
                    for ob in range(NDC):
                        ph = wp.tile([NP, HTS], F32, tag="m", bufs=2)
                        for c2 in range(NDC):
                            nc.tensor.matmul(
                                ph[:], wot_sb[:, 2 * c2:2 * c2 + 2,
                                              ob * NP:(ob + 1) * NP],
                                ao[:, 2 * c2:2 * c2 + 2, :],
                                start=(c2 == 0), stop=(c2 == NDC - 1),
                                perf_mode=DR)
                        hb = sbo.tile([NP, HTS], F32, tag="hb", bufs=2,
                                      name=f"hb{h01}_{ob}")
                        nc.scalar.activation(hb[:], ph[:], AF.Identity,
                                             bias=bo_sb[:, ob:ob + 1],
                                             scale=1.0 / (SW * SW))
                        nc.vector.tensor_tensor(
                            sum1[:, ob, :], hb[:], xtr_sb[:, ob, hsl],
                            op=ALU.add)
                    return sum1

                def ln_sq(pool, src, W, pre=None):
                    sqb = pool.tile([NP, NDC, W], F32R, tag="ln_sq", bufs=2)
                    for ob in range(NDC):
                        if pre is not None:
                            pre(ob)
                        eng = nc.gpsimd if ob % 2 else nc.vector
                        eng.tensor_tensor(
                            sqb[:, ob, :], src[:, ob, :], src[:, ob, :],
                            op=ALU.mult)
                    return sqb

                def ln_rows(pool, lp, src, sqb, W):
                    pmu = lp.tile([1, W], F32, tag="statmu")
                    psq = lp.tile([1, W], F32, tag="statsq")
                    pmu, psq = pmu[:], psq[:]
                    for ob in range(NDC):
                        nc.tensor.matmul(pmu, onescol_r[:], src[:, ob, :],
                                         start=(ob == 0), stop=(ob == NDC - 1))
                        nc.tensor.matmul(psq, onescol_r[:], sqb[:, ob, :],
                                         start=(ob == 0), stop=(ob == NDC - 1))
                    rows = pool.tile([1, 4, W], F32, tag="ln_rows", bufs=2)
                    mu, ex2e, mu2, vr = (rows[:, i, :] for i in range(4))
                    nc.vector.tensor_scalar_mul(mu, pmu, 1.0 / D)
                    nc.vector.tensor_scalar(ex2e, psq, 1.0 / D, EPS,
                                            op0=ALU.mult, op1=ALU.add)
                    nc.vector.tensor_tensor(mu2, mu, mu, op=ALU.mult)
                    nc.vector.tensor_tensor(mu2, ex2e, mu2, op=ALU.subtract)
                    nc.vector.reciprocal(vr, mu2)
                    rowr = pool.tile([1, 2, W], F32R, tag="ln_rowr", bufs=2)
                    nc.scalar.activation(rowr[:, 0, :], vr, AF.Sqrt)
                    nc.vector.scalar_tensor_tensor(
                        rowr[:, 1, :], mu, -1.0, rowr[:, 0, :],
                        op0=ALU.mult, op1=ALU.mult)
                    return rowr

                def ln_finish(pool, lp, src, rowr, outf, outq, W, qoff=None):
                    pA = lp.tile([NP, W], F32, tag="bA")
                    pB = lp.tile([NP, W], F32, tag="bB")
                    pA, pB = pA[:], pB[:]
                    nc.tensor.matmul(pA, ones128r[:], rowr[:, 0, :],
                                     start=True, stop=True)
                    nc.tensor.matmul(pB, ones128r[:], rowr[:, 1, :],
                                     start=True, stop=True)
                    A_sb = pool.tile([NP, W], F32, tag="ln_A", bufs=2)
                    B_sb = pool.tile([NP, W], F32, tag="ln_B", bufs=2)
                    nc.scalar.activation(A_sb[:], pA, AF.Copy)
                    nc.scalar.activation(B_sb[:], pB, AF.Copy)
                    tmp = pool.tile([NP, NDC, W], F32, tag="ln_tmp", bufs=2)
                    for ob in range(NDC):
                        eng = nc.gpsimd if ob % 3 == 2 else nc.vector
                        eng.tensor_tensor(
                            tmp[:, ob, :], src[:, ob, :], A_sb[:],
                            op=ALU.mult)
                        nc.vector.tensor_tensor(
                            outf[ob], tmp[:, ob, :], B_sb[:], op=ALU.add)
                        if outq is not None:
                            eng2 = nc.gpsimd if ob % 3 == 1 else nc.vector
                            eng2.tensor_copy(outq[ob], outf[ob])

                with tc.tile_pool(name="sbo", bufs=1) as sbo, \
                     tc.tile_pool(name="ln1p", bufs=1) as lnp, \
                     tc.tile_pool(name="ln1_ps", bufs=1, space="PSUM") as lp, \
                     tc.tile_pool(name="wo_ps", bufs=1, space="PSUM") as wp:
                    for h01 in range(2):
                        sum1 = wo_half(h01, wp, sbo)
                        sqb = ln_sq(lnp, sum1, HTS)
                        rowr = ln_rows(lnp, lp, sum1, sqb, HTS)
                        ln_finish(lnp, lp, sum1, rowr,
                                  [x2f[:, ob, hs[h01]] for ob in range(NDC)],
                                  [x2q[:, ob, hs[h01]] for ob in range(NDC)],
                                  HTS)

                # W1 + gelu (full-width, fp8 DR over dc pairs)
                with tc.tile_pool(name="w1s", bufs=1) as w1s, \
                     tc.tile_pool(name="w1_ps", bufs=1, space="PSUM") as mp:
                    for fc in range(FFN // QT):
                        w1c = w1s.tile([NP, NDC, QT], FP8, tag="w1c",
                                       bufs=3, name=f"w1c{fc}")
                        nc.sync.dma_start(
                            out=w1c[:],
                            in_=w1t_d[:, fc * QT:(fc + 1) * QT].rearrange(
                                "(c p) f -> p c f", p=NP))
                        for fs in range(QT // NP):
                            fb = fc * (QT // NP) + fs
                            pm = mp.tile([NP, TS], F32, tag="pm", bufs=3)
                            for p4 in range(NP4):
                                nc.tensor.matmul(
                                    pm[:],
                                    w1c[:, 2 * p4:2 * p4 + 2,
                                        fs * NP:(fs + 1) * NP],
                                    x2q[:, 2 * p4:2 * p4 + 2, :],
                                    start=(p4 == 0), stop=(p4 == NP4 - 1),
                                    perf_mode=DR)
                            nc.scalar.activation(
                                g_all[:, fb, :], pm[:],
                                AF.Sigmoid if for_sim else AF.Gelu_apprx_tanh,
                                bias=b1_sb[:, fb:fb + 1], scale=1.0 / SW)

                # W2 in two half-token passes + LN2 overlapped
                with tc.tile_pool(name="w2p", bufs=1) as w2p, \
                     tc.tile_pool(name="ln2", bufs=1) as lnp2:
                    sum2 = w2p.tile([NP, NDC, TS], F32R)
                    yf = w2p.tile([NP, NDC, TS], F32)
                    yp_cm = tc.tile_pool(name="w2_ps", bufs=1, space="PSUM")
                    yp = yp_cm.__enter__()
                    for h01 in range(2):
                        hsl = hs[h01]
                        # one accumulator tile (= one PSUM bank) per output
                        # block: matmul start=True zeroes whole 2KB banks,
                        # so accumulators must never share a bank. 4 tags
                        # reused across ob-groups/halves keep stats banks
                        # free for the overlapped LN2.
                        ys_all = {}
                        for obg in range(2):
                            ys_t = [yp.tile([NP, HTS], F32, tag=f"ys{ob4}",
                                            bufs=1, name=f"ys{h01}_{obg}_{ob4}")
                                    for ob4 in range(4)]
                            for fp8i in range(FFN // NP // 2):
                                for ob4 in range(4):
                                    ob = obg * 4 + ob4
                                    nc.tensor.matmul(
                                        ys_t[ob4][:],
                                        w2_sb[:, fp8i, :,
                                              ob * NP:(ob + 1) * NP],
                                        g_all[:, 2 * fp8i:2 * fp8i + 2, hsl],
                                        start=(fp8i == 0),
                                        stop=(fp8i == FFN // NP // 2 - 1),
                                        perf_mode=DR)
                            for ob4 in range(4):
                                ys_all[obg * 4 + ob4] = ys_t[ob4]

                        def w2_tail(ob, h01=h01, ys_all=ys_all):
                            mb = w2p.tile([NP, HTS], F32, tag="mb", bufs=2,
                                          name=f"mb{h01}_{ob}")
                            nc.scalar.activation(
                                mb[:], ys_all[ob][:], AF.Identity,
                                bias=b2_sb[:, ob:ob + 1], scale=1.0 / SW2)
                            nc.vector.tensor_tensor(
                                sum2[:, ob, hs[h01]], mb[:],
                                x2f[:, ob, hs[h01]], op=ALU.add)

                        src = sum2[:, :, hsl]
                        sqb = ln_sq(lnp2, src, HTS, pre=w2_tail)
                        with tc.tile_pool(name=f"ln2_ps{h01}", bufs=1,
                                          space="PSUM") as lp2:
                            rowr = ln_rows(lnp2, lp2, src, sqb, HTS)
                            ln_finish(lnp2, lp2, src, rowr,
                                      [yf[:, ob, hsl] for ob in range(NDC)],
                                      None, HTS)
                        for ob in range(NDC):
                            nc.sync.dma_start(
                                out=out_d[ob * NP:(ob + 1) * NP, hsl],
                                in_=yf[:, ob, hsl])
                    yp_cm.__exit__(None, None, None)
            prefetch.__exit__(None, None, None)

    if split:
        _split_multiwait_instructions(nc)
    return nc


# ---------------------------------------------------------------------------
# Cached PJRT runner (mirrors bass2jax.run_bass_via_pjrt multi-core path but
# keeps the jitted callable so repeat calls don't recompile).
# ---------------------------------------------------------------------------

_RUNNER = None


def _make_runner(nc):
    import jax
    from jax.sharding import Mesh, PartitionSpec
    from jax.experimental.shard_map import shard_map
    from concourse import bass2jax

    bass2jax.install_neuronx_cc_hook()
    partition_name = (nc.partition_id_tensor.name
                      if nc.partition_id_tensor else None)
    in_names, out_names, out_avals = [], [], []
    for alloc in nc.m.functions[0].allocations:
        if not isinstance(alloc, mybir.MemoryLocationSet):
            continue
        name = alloc.memorylocations[0].name
        if alloc.kind == "ExternalInput":
            if name != partition_name:
                in_names.append(name)
        elif alloc.kind == "ExternalOutput":
            out_names.append(name)
            out_avals.append(jax.core.ShapedArray(
                tuple(alloc.tensor_shape), mybir.dt.np(alloc.dtype)))
    n_params = len(in_names)
    all_in_names = list(in_names) + list(out_names)
    if partition_name is not None:
        all_in_names.append(partition_name)
    donate = tuple(range(n_params, n_params + len(out_names)))

    def _body(*args):
        operands = list(args)
        if partition_name is not None:
            operands.append(bass2jax.partition_id_tensor())
        outs = bass2jax._bass_exec_p.bind(
            *operands,
            out_avals=tuple(out_avals),
            in_names=tuple(all_in_names),
            out_names=tuple(out_names),
            lowering_input_output_aliases=(),
            sim_require_finite=True,
            sim_require_nnan=True,
            nc=nc,
        )
        return tuple(outs)

    devices = jax.devices()[:NCORES]
    mesh = Mesh(np.asarray(devices), ("core",))
    specs = (PartitionSpec("core"),) * (n_params + len(out_names))
    sharded = jax.jit(
        shard_map(_body, mesh=mesh, in_specs=specs,
                  out_specs=(PartitionSpec("core"),) * len(out_names),
                  check_rep=False),
        donate_argnums=donate, keep_unused=True)

    from jax.sharding import NamedSharding
    shard = NamedSharding(mesh, PartitionSpec("core"))

    def prepare(in_maps):
        per_core = [[np.asarray(m[name]) for name in in_names]
                    for m in in_maps]
        concat_in = [np.concatenate([per_core[c][i] for c in range(NCORES)],
                                    axis=0) for i in range(n_params)]
        return [jax.device_put(a, shard) for a in concat_in]

    def run_prepared(dev_in, materialize=True):
        concat_zeros = [np.zeros((NCORES * a.shape[0], *a.shape[1:]), a.dtype)
                        for a in out_avals]
        out_arrs = sharded(*dev_in, *concat_zeros)
        if not materialize:
            jax.block_until_ready(out_arrs)
            return None
        return [
            {name: np.asarray(out_arrs[i]).reshape(
                NCORES, *out_avals[i].shape)[c]
             for i, name in enumerate(out_names)}
            for c in range(NCORES)
        ]

    def run_prepared_async(dev_in):
        concat_zeros = [np.zeros((NCORES * a.shape[0], *a.shape[1:]), a.dtype)
                        for a in out_avals]
        return sharded(*dev_in, *concat_zeros)

    def run(in_maps, materialize=True):
        return run_prepared(prepare(in_maps), materialize)

    run.prepare = prepare
    run.run_prepared = run_prepared
    run.run_prepared_async = run_prepared_async
    return run


def _get_runner():
    global _RUNNER
    if _RUNNER is None:
        nc = _build_program()
        _RUNNER = _make_runner(nc)
    return _RUNNER


# ---------------------------------------------------------------------------
# Host-side sharding / gathering
# ---------------------------------------------------------------------------

def _fp8(a):
    return np.ascontiguousarray(a).astype(ml_dtypes.float8_e4m3)


def make_in_maps(tgt, tgt_mask, tgt_key_pad_mask, Wq, Wk, Wv, Wo, bo,
                 W1, b1, W2, b2, g1, beta1, g2, beta2):
    causal = np.triu(np.ones((N, N), bool), k=1)
    if not (np.array_equal(np.asarray(tgt_mask), causal)
            and not np.asarray(tgt_key_pad_mask).any()
            and np.allclose(np.asarray(g1), 1) and np.allclose(np.asarray(g2), 1)
            and np.allclose(np.asarray(beta1), 0)
            and np.allclose(np.asarray(beta2), 0)):
        return None  # unsupported masking/affine -> numpy fallback

    tgt = np.asarray(tgt, np.float32)
    Wq, Wk, Wv, Wo = (np.asarray(a, np.float32) for a in (Wq, Wk, Wv, Wo))
    W1, W2 = np.asarray(W1, np.float32), np.asarray(W2, np.float32)
    bo, b1, b2 = (np.asarray(a, np.float32) for a in (bo, b1, b2))

    # single [128, 128] diagonal-triangle mask (same for every block)
    maskq = (np.arange(NP)[:, None] <= np.arange(NP)[None, :]).astype(
        np.float32)

    w1t = _fp8(SW * W1.T)
    w2t = _fp8(SW2 * W2.T)
    bo_c = np.ascontiguousarray(bo.reshape(D // NP, NP).T)
    b1_c = np.ascontiguousarray(b1.reshape(FFN // NP, NP).T)
    b2_c = np.ascontiguousarray(b2.reshape(D // NP, NP).T)
    maskq8 = _fp8(maskq)

    xt_b = [_fp8(tgt[b].T) for b in range(B)]
    in_maps = []
    for g in range(NCORES):
        b, r = g // TPR, g % TPR
        sl = slice(DVL * r, DVL * (r + 1))
        wot_ext = np.zeros((2 * D, D), np.float32)
        wot_ext[D * b:D * (b + 1), :] = SW * Wo.T
        in_maps.append({
            "xt": xt_b[b],
            "xtr": np.ascontiguousarray(tgt[b].T[:, TS * r:TS * (r + 1)]),
            "wqt": _fp8(SW * Wq[sl, :].T),
            "wkt": _fp8(SW * Wk[sl, :].T),
            "wvt": _fp8(SW * Wv[sl, :].T),
            "wot": _fp8(wot_ext),
            "w1t": w1t,
            "w2t": w2t,
            "bo": bo_c,
            "b1": b1_c,
            "b2": b2_c,
            "maskq": maskq8,
        })
    return in_maps


def _numpy_reference(tgt, tgt_mask, tgt_key_pad_mask, Wq, Wk, Wv, Wo, bo,
                     W1, b1, W2, b2, g1, beta1, g2, beta2):
    def ln(x, g, b):
        mu = x.mean(-1, keepdims=True)
        var = ((x - mu) ** 2).mean(-1, keepdims=True)
        return (x - mu) / np.sqrt(var + EPS) * g + b

    x = np.asarray(tgt, np.float64)
    b_, n, d = x.shape
    dk = d // H
    q = (x @ Wq.T).reshape(b_, n, H, dk).transpose(0, 2, 1, 3)
    k = (x @ Wk.T).reshape(b_, n, H, dk).transpose(0, 2, 1, 3)
    v = (x @ Wv.T).reshape(b_, n, H, dk).transpose(0, 2, 1, 3)
    s = np.einsum("bhqd,bhkd->bhqk", q, k) / np.sqrt(dk)
    mask = np.asarray(tgt_mask)[None, None] | \
        np.asarray(tgt_key_pad_mask)[:, None, None, :]
    s = np.where(mask, -np.inf, s)
    s = s - s.max(-1, keepdims=True)
    e = np.exp(s)
    att = e / e.sum(-1, keepdims=True)
    o = np.einsum("bhqk,bhkd->bhqd", att, v).transpose(0, 2, 1, 3).reshape(
        b_, n, d)
    h = o @ Wo.T + bo
    x1 = ln(x + h, g1, beta1)
    gl = x1 @ W1.T + b1
    gl = 0.5 * gl * (1 + np.tanh(np.sqrt(2 / np.pi) * (gl + 0.044715 * gl**3)))
    m = gl @ W2.T + b2
    return ln(x1 + m, g2, beta2).astype(np.float32)


def kernel(**inputs):
    in_maps = make_in_maps(**inputs)
    if in_maps is None:
        return _numpy_reference(**inputs)
    run = _get_runner()
    results = run(in_maps)
    out = np.empty((B, N, D), np.float32)
    for g in range(NCORES):
        b, r = g // TPR, g % TPR
        out[b, TS * r:TS * (r + 1), :] = results[g]["out"].T
    return out
